# revision 16
# baseline (speedup 1.0000x reference)
"""DividedAttention (TimeSformer-style divided space-time attention) on 8 trn2 cores.

Sharding: pure data-parallel over batch B=16 -> 2 batch items per core.
Per-core pipeline (bf16 matmuls, fp32 accumulation):
  host: x -> xT (pre-transposed, bf16), W_qkv (q part pre-scaled by dh^-0.5), W_out
  QK^T projection (out^T form) -> qT/kT per head, frame-aligned, CLS query
    replicated as q column 196, CLS key as k column 0 of every frame
  V projection (natural form) -> v_flat -> frame-aligned v_fra/v_frb via sbuf DMA
  per head-pair: S^T = (kT chunk)^T @ qT -> exp -> E^T (bf16, AV-ready);
    column sums via ones-matmuls; AV = V^T-chunks @ E^T -> O^T in psum;
    normalization: PE-broadcast of the sum rows -> DVE reciprocal -> fused
    multiply into the attnT write. CLS partials ride along as q column 196.
  out-projection from attnT + bias -> DMA out
"""
import sys

sys.path.insert(0, "/opt/trn_rl_repo")

import numpy as np
import ml_dtypes

from concourse import bacc
import concourse.mybir as mybir
import concourse.tile as tile
from concourse import bass_utils

BF16 = mybir.dt.bfloat16
F32 = mybir.dt.float32
NPBF = ml_dtypes.bfloat16

B, SP, F, DIM, H, DH = 16, 196, 8, 512, 8, 64
INNER = H * DH            # 512
N = 1 + F * SP            # 1569
SP1 = SP + 1              # 197
NCORES = 8
NB = B // NCORES          # 2
KC = DIM // 128           # 4
NT = (N + 127) // 128     # 13
LAST = N - 128 * (NT - 1)  # 33
TCH = [(0, 1 + 2 * SP), (1 + 2 * SP, 2 * SP), (1 + 4 * SP, 2 * SP), (1 + 6 * SP, 2 * SP)]

EXP = mybir.ActivationFunctionType.Exp
ADD = mybir.AluOpType.add
MULT = mybir.AluOpType.mult


def _v_pieces(tok0, length):
    out = []
    done = 0
    while done < length:
        tok = tok0 + done
        t, p0 = divmod(tok, 128)
        l = min(128 - p0, length - done)
        out.append((t, p0, done, l))
        done += l
    return out


def build_nc():
    nc = bacc.Bacc(num_devices=NCORES)

    xT = nc.declare_dram_parameter("xT", [NB, DIM, N], BF16, isOutput=False)
    wqkv = nc.declare_dram_parameter("wqkv", [DIM, 3 * INNER], BF16, isOutput=False)
    wout = nc.declare_dram_parameter("wout", [INNER, DIM], BF16, isOutput=False)
    bout = nc.declare_dram_parameter("bout", [1, DIM], F32, isOutput=False)
    out = nc.declare_dram_parameter("out", [NB, N, DIM], F32, isOutput=True)

    with tile.TileContext(nc) as tc:
        with (
            tc.tile_pool(name="const", bufs=1) as const,
            tc.tile_pool(name="big", bufs=1) as big,
            tc.tile_pool(name="big2", bufs=2) as big2,
            tc.tile_pool(name="xt", bufs=2) as xt_pool,
            tc.tile_pool(name="ebuf", bufs=3) as ebuf,
            tc.tile_pool(name="small", bufs=3) as small,
            tc.tile_pool(name="outp", bufs=3) as outp,
            tc.tile_pool(name="ps_proj", bufs=2, space="PSUM") as ps_proj,
            tc.tile_pool(name="ps_s", bufs=2, space="PSUM") as ps_s,
            tc.tile_pool(name="ps_o", bufs=1, space="PSUM") as ps_o,
        ):
            # ---- constants
            wqkv_sb = const.tile([128, KC, 3 * INNER], BF16)
            nc.sync.dma_start(out=wqkv_sb, in_=wqkv[:, :].rearrange("(c p) o -> p c o", p=128))
            wout_sb = const.tile([128, KC, DIM], BF16)
            nc.sync.dma_start(out=wout_sb, in_=wout[:, :].rearrange("(c p) o -> p c o", p=128))
            bout_sb = const.tile([128, DIM], F32)
            nc.sync.dma_start(out=bout_sb, in_=bout[:, :].to_broadcast([128, DIM]))
            ones_row = const.tile([1, 64], BF16)
            nc.vector.memset(ones_row, 1.0)
            ones128 = const.tile([128, 64], BF16)
            nc.vector.memset(ones128, 1.0)

            for b in range(NB):
                # ---- load xT
                xT_sb = xt_pool.tile([128, KC, N], BF16, tag="xt")
                for kc in range(KC):
                    nc.sync.dma_start(out=xT_sb[:, kc, :],
                                      in_=xT[b, 128 * kc:128 * (kc + 1), :])

                # ---- per-batch sbuf tensors
                qT_fr = big2.tile([128, 4, F, SP1], BF16, tag="qT")
                kT_fr = big2.tile([128, 4, F, SP1], BF16, tag="kT")
                v_flat = big.tile([128, NT, INNER], BF16, tag="vflat")
                v_fra = big2.tile([128, F, INNER], BF16, tag="vfra")
                v_frb = big2.tile([128, F, INNER], BF16, tag="vfrb")
                attnT = big.tile([128, KC, N], BF16, tag="attnT")
                # CLS bookkeeping
                oscls_acc = big.tile([128, 2, 4], F32, tag="oclsacc")
                ocls_acc = oscls_acc[:, 0, :]
                scls_acc = oscls_acc[:, 1, :]
                ecc_bc = big.tile([128, 4], F32, tag="eccbc")
                vTcls = big.tile([128, 4], F32, tag="vTcls")
                ecc_row = big.tile([1, H], BF16, tag="eccrow")
                rcls_bc = big.tile([128, 4], F32, tag="rclsbc")
                t_evc = big.tile([128, 4], F32, tag="tevc")
                t_corr = big.tile([128, 4], F32, tag="tcorr")

                nc.vector.memset(oscls_acc, 0.0)

                # ---- Q/K projection (out^T form)
                for oc in range(8):
                    isq = oc < 4
                    hp = oc if isq else oc - 4
                    for ci, (t0, tl) in enumerate(TCH):
                        ps = ps_proj.tile([128, 512], F32, tag="proj")
                        for kc in range(KC):
                            nc.tensor.matmul(
                                ps[:, :tl],
                                lhsT=wqkv_sb[:, kc, oc * 128:(oc + 1) * 128],
                                rhs=xT_sb[:, kc, t0:t0 + tl],
                                start=(kc == 0),
                                stop=(kc == KC - 1),
                            )
                        dst = qT_fr if isq else kT_fr
                        eng = nc.scalar if isq else nc.vector
                        cp = eng.copy if isq else eng.tensor_copy
                        o0 = 0 if isq else 1
                        if ci == 0:
                            cp(
                                dst[:, hp, 0:2, o0:o0 + SP],
                                ps[:, 1:tl].rearrange("p (a s) -> p a s", a=2),
                            )
                            ccol = SP if isq else 0
                            cp(
                                dst[:, hp, 0:F, ccol:ccol + 1],
                                ps[:, None, 0:1].to_broadcast([128, F, 1]),
                            )
                        else:
                            cp(
                                dst[:, hp, 2 * ci:2 * ci + 2, o0:o0 + SP],
                                ps[:, :tl].rearrange("p (a s) -> p a s", a=2),
                            )

                # ---- V projection (natural form) -> v_flat
                for t in range(NT):
                    m = 128 if t < NT - 1 else LAST
                    ps = ps_proj.tile([128, 512], F32, tag="proj")
                    for kc in range(KC):
                        nc.tensor.matmul(
                            ps[:m, :],
                            lhsT=xT_sb[:, kc, 128 * t:128 * t + m],
                            rhs=wqkv_sb[:, kc, 2 * INNER:3 * INNER],
                            start=(kc == 0),
                            stop=(kc == KC - 1),
                        )
                    nc.scalar.copy(v_flat[:m, t, :], ps[:m, :])

                # vT_cls [d x 2heads, hp] via tiny K=1 transposes of the V cls row
                pvt = ps_proj.tile([128, 512], F32, tag="proj")
                for hp in range(4):
                    nc.tensor.matmul(
                        pvt[:, hp:hp + 1],
                        lhsT=v_flat[0:1, 0, 128 * hp:128 * (hp + 1)],
                        rhs=ones_row[0:1, 0:1],
                        start=True, stop=True,
                    )
                nc.vector.tensor_copy(vTcls, pvt[:, 0:4])

                # ---- V rearrange to frame-aligned layouts (sbuf->sbuf DMA)
                nc.sync.dma_start(
                    out=v_fra[0:1, 0:F, :],
                    in_=v_flat[0:1, 0, None, :].to_broadcast([1, F, INNER]),
                )
                for f in range(F):
                    for (t, p0, d0, l) in _v_pieces(1 + SP * f, 127):
                        nc.sync.dma_start(
                            out=v_fra[1 + d0:1 + d0 + l, f, :],
                            in_=v_flat[p0:p0 + l, t, :],
                        )
                    for (t, p0, d0, l) in _v_pieces(128 + SP * f, 69):
                        nc.sync.dma_start(
                            out=v_frb[d0:d0 + l, f, :],
                            in_=v_flat[p0:p0 + l, t, :],
                        )

                # ---- attention per head-pair (S^T pipeline)
                for hp in range(4):
                    ea = ebuf.tile([128, 2, F, SP1], BF16, tag="ea")
                    eb = ebuf.tile([128, 2, F, SP1], BF16, tag="eb")

                    def pass1(f):
                        st_a = ps_s.tile([128, 2, 512], F32, tag="s")
                        st_b = ps_s.tile([128, 2, 512], F32, tag="s")
                        for par in range(2):
                            rows = slice(64 * par, 64 * par + 64)
                            nc.tensor.matmul(
                                st_a[:, par, 0:SP1],
                                lhsT=kT_fr[rows, hp, f, 0:128],
                                rhs=qT_fr[rows, hp, f, :],
                                start=True, stop=True,
                            )
                            nc.tensor.matmul(
                                st_b[0:69, par, 0:SP1],
                                lhsT=kT_fr[rows, hp, f, 128:SP1],
                                rhs=qT_fr[rows, hp, f, :],
                                start=True, stop=True,
                            )
                        nc.scalar.activation(ea[:, :, f, :], st_a[:, :, 0:SP1], EXP)
                        nc.scalar.activation(eb[0:69, :, f, :], st_b[0:69, :, 0:SP1], EXP)

                    def pass2(f):
                        po = ps_o.tile([128, 1024], F32, tag="o")
                        for par in range(2):
                            h = 2 * hp + par
                            rows = slice(64 * par, 64 * par + 64)
                            hs = slice(DH * h, DH * (h + 1))
                            nc.tensor.matmul(
                                po[rows, 0:SP1],
                                lhsT=v_fra[:, f, hs],
                                rhs=ea[:, par, f, :],
                                start=True, stop=False,
                            )
                            nc.tensor.matmul(
                                po[rows, 0:SP1],
                                lhsT=v_frb[0:69, f, hs],
                                rhs=eb[0:69, par, f, :],
                                start=False, stop=True,
                            )
                        # column sums broadcast per parity (rows match parity)
                        for par in range(2):
                            rows = slice(64 * par, 64 * par + 64)
                            nc.tensor.matmul(
                                po[rows, 512:512 + SP1],
                                lhsT=ones128[:, 0:64],
                                rhs=ea[:, par, f, :],
                                start=True, stop=False,
                            )
                            nc.tensor.matmul(
                                po[rows, 512:512 + SP1],
                                lhsT=ones128[0:69, 0:64],
                                rhs=eb[0:69, par, f, :],
                                start=False, stop=True,
                            )
                        rbc = small.tile([128, SP1], F32, tag="rbc")
                        nc.vector.reciprocal_approx_fast(rbc, po[:, 512:512 + SP1])
                        nc.vector.tensor_tensor(
                            attnT[:, hp, 1 + SP * f:1 + SP * (f + 1)],
                            po[:, 0:SP], rbc[:, 0:SP], MULT,
                        )
                        nc.vector.tensor_tensor(
                            oscls_acc[:, :, hp], po[:, SP:SP + 513:512],
                            oscls_acc[:, :, hp], ADD,
                        )

                    LAG = 2
                    for f in range(F + LAG):
                        if f < F:
                            pass1(f)
                        if f >= LAG:
                            pass2(f - LAG)

                    # e_cc for this head pair (row space)
                    for par in range(2):
                        h = 2 * hp + par
                        nc.scalar.copy(ecc_row[0:1, h:h + 1], ea[0:1, par, 0, SP:SP1])

                # ---- CLS finalization (bc space)
                pec = ps_proj.tile([128, 512], F32, tag="proj")
                for hp in range(4):
                    for par in range(2):
                        h = 2 * hp + par
                        rows = slice(64 * par, 64 * par + 64)
                        nc.tensor.matmul(pec[rows, hp:hp + 1], lhsT=ones_row,
                                         rhs=ecc_row[0:1, h:h + 1],
                                         start=True, stop=True)
                nc.vector.tensor_copy(ecc_bc, pec[:, 0:4])
                # denominator: scls_acc - 7 e_cc -> reciprocal
                nc.vector.scalar_tensor_tensor(
                    scls_acc, ecc_bc, -7.0, scls_acc, op0=MULT, op1=ADD,
                )
                nc.vector.reciprocal_approx_fast(rcls_bc, scls_acc)
                # numerator: ocls_acc - 7 e_cc * vTcls, then normalize
                nc.vector.tensor_tensor(t_evc, ecc_bc, vTcls, MULT)
                nc.vector.scalar_tensor_tensor(
                    t_corr, t_evc, -7.0, ocls_acc, op0=MULT, op1=ADD,
                )
                nc.vector.tensor_tensor(t_corr, t_corr, rcls_bc, MULT)
                nc.vector.tensor_copy(attnT[:, 0:4, 0:1], t_corr[:, :, None])

                # ---- out projection + bias (token tile 0 last: waits on CLS)
                for t in list(range(1, NT)) + [0]:
                    m = 128 if t < NT - 1 else LAST
                    ps = ps_proj.tile([128, 512], F32, tag="proj")
                    for kc in range(KC):
                        nc.tensor.matmul(
                            ps[:m, :],
                            lhsT=attnT[:, kc, 128 * t:128 * t + m],
                            rhs=wout_sb[:, kc, :],
                            start=(kc == 0),
                            stop=(kc == KC - 1),
                        )
                    osb = outp.tile([128, DIM], F32, tag="out")
                    nc.vector.tensor_tensor(osb[:m, :], ps[:m, :], bout_sb[:m, :], ADD)
                    nc.sync.dma_start(out=out[b, 128 * t:128 * t + m, :], in_=osb[:m, :])

    nc.finalize()
    return nc


_CACHE = {}


def _get_nc():
    if "nc" not in _CACHE:
        _CACHE["nc"] = build_nc()
    return _CACHE["nc"]


def prepare_in_maps(x, f, W_qkv, W_out, b_out):
    assert int(f) == F
    x = np.asarray(x, dtype=np.float32)
    W_qkv = np.asarray(W_qkv, dtype=np.float32).copy()
    W_out = np.asarray(W_out, dtype=np.float32)
    b_out = np.asarray(b_out, dtype=np.float32)
    W_qkv[:, :INNER] *= DH ** -0.5
    wqkv_bf = W_qkv.astype(NPBF)
    wout_bf = W_out.astype(NPBF)
    bout_np = b_out.reshape(1, DIM)
    xT = np.ascontiguousarray(x.transpose(0, 2, 1)).astype(NPBF)
    in_maps = []
    for c in range(NCORES):
        in_maps.append({
            "xT": np.ascontiguousarray(xT[NB * c:NB * (c + 1)]),
            "wqkv": wqkv_bf,
            "wout": wout_bf,
            "bout": bout_np,
        })
    return in_maps


def kernel(x, f, W_qkv, W_out, b_out):
    nc = _get_nc()
    in_maps = prepare_in_maps(x, f, W_qkv, W_out, b_out)
    res = bass_utils.run_bass_kernel_spmd(nc, in_maps, list(range(NCORES)))
    return np.concatenate([r["out"] for r in res.results], axis=0)


# revision 18
# speedup vs baseline: 1.0102x; 1.0102x over previous
"""DividedAttention (TimeSformer-style divided space-time attention) on 8 trn2 cores.

Sharding: pure data-parallel over batch B=16 -> 2 batch items per core.
Per-core pipeline (bf16 matmuls, fp32 accumulation):
  host: x -> xT (pre-transposed, bf16), W_qkv (q part pre-scaled by dh^-0.5), W_out
  QK^T projection (out^T form) -> qT/kT per head, frame-aligned, CLS query
    replicated as q column 196, CLS key as k column 0 of every frame
  V projection (natural form) -> v_flat -> frame-aligned v_fra/v_frb via sbuf DMA
  per head-pair: S^T = (kT chunk)^T @ qT -> exp -> E^T (bf16, AV-ready);
    column sums via ones-matmuls; AV = V^T-chunks @ E^T -> O^T in psum;
    normalization: PE-broadcast of the sum rows -> DVE reciprocal -> fused
    multiply into the attnT write. CLS partials ride along as q column 196.
  out-projection from attnT + bias -> DMA out
"""
import sys

sys.path.insert(0, "/opt/trn_rl_repo")

import numpy as np
import ml_dtypes

from concourse import bacc
import concourse.mybir as mybir
import concourse.tile as tile
from concourse import bass_utils

BF16 = mybir.dt.bfloat16
F32 = mybir.dt.float32
NPBF = ml_dtypes.bfloat16

B, SP, F, DIM, H, DH = 16, 196, 8, 512, 8, 64
INNER = H * DH            # 512
N = 1 + F * SP            # 1569
SP1 = SP + 1              # 197
NCORES = 8
NB = B // NCORES          # 2
KC = DIM // 128           # 4
NT = (N + 127) // 128     # 13
LAST = N - 128 * (NT - 1)  # 33
TCH = [(0, 1 + 2 * SP), (1 + 2 * SP, 2 * SP), (1 + 4 * SP, 2 * SP), (1 + 6 * SP, 2 * SP)]

EXP = mybir.ActivationFunctionType.Exp
ADD = mybir.AluOpType.add
MULT = mybir.AluOpType.mult


def _v_pieces(tok0, length):
    out = []
    done = 0
    while done < length:
        tok = tok0 + done
        t, p0 = divmod(tok, 128)
        l = min(128 - p0, length - done)
        out.append((t, p0, done, l))
        done += l
    return out


def build_nc():
    nc = bacc.Bacc(num_devices=NCORES)

    xT = nc.declare_dram_parameter("xT", [NB, DIM, N], BF16, isOutput=False)
    wqkv = nc.declare_dram_parameter("wqkv", [DIM, 3 * INNER], BF16, isOutput=False)
    wout = nc.declare_dram_parameter("wout", [INNER, DIM], BF16, isOutput=False)
    bout = nc.declare_dram_parameter("bout", [1, DIM], F32, isOutput=False)
    out = nc.declare_dram_parameter("out", [NB, N, DIM], F32, isOutput=True)

    with tile.TileContext(nc) as tc:
        with (
            tc.tile_pool(name="const", bufs=1) as const,
            tc.tile_pool(name="big", bufs=1) as big,
            tc.tile_pool(name="big2", bufs=2) as big2,
            tc.tile_pool(name="xt", bufs=2) as xt_pool,
            tc.tile_pool(name="ebuf", bufs=3) as ebuf,
            tc.tile_pool(name="small", bufs=3) as small,
            tc.tile_pool(name="outp", bufs=3) as outp,
            tc.tile_pool(name="ps_proj", bufs=2, space="PSUM") as ps_proj,
            tc.tile_pool(name="ps_s", bufs=2, space="PSUM") as ps_s,
            tc.tile_pool(name="ps_o", bufs=1, space="PSUM") as ps_o,
        ):
            # ---- constants
            wqkv_sb = const.tile([128, KC, 3 * INNER], BF16)
            nc.sync.dma_start(out=wqkv_sb, in_=wqkv[:, :].rearrange("(c p) o -> p c o", p=128))
            wout_sb = const.tile([128, KC, DIM], BF16)
            nc.sync.dma_start(out=wout_sb, in_=wout[:, :].rearrange("(c p) o -> p c o", p=128))
            bout_sb = const.tile([128, DIM], F32)
            nc.sync.dma_start(out=bout_sb, in_=bout[:, :].to_broadcast([128, DIM]))
            ones_row = const.tile([1, 64], BF16)
            nc.vector.memset(ones_row, 1.0)
            ones128 = const.tile([128, 64], BF16)
            nc.vector.memset(ones128, 1.0)

            for b in range(NB):
                # ---- load xT
                xT_sb = xt_pool.tile([128, KC, N], BF16, tag="xt")
                for kc in range(KC):
                    nc.sync.dma_start(out=xT_sb[:, kc, :],
                                      in_=xT[b, 128 * kc:128 * (kc + 1), :])

                # ---- per-batch sbuf tensors
                qT_fr = big2.tile([128, 4, F, SP1], BF16, tag="qT")
                kT_fr = big2.tile([128, 4, F, SP1], BF16, tag="kT")
                v_flat = big.tile([128, NT, INNER], BF16, tag="vflat")
                v_fra = big2.tile([128, F, INNER], BF16, tag="vfra")
                v_frb = big2.tile([128, F, INNER], BF16, tag="vfrb")
                attnT = big.tile([128, KC, N], BF16, tag="attnT")
                # CLS bookkeeping
                oscls_acc = big.tile([128, 2, 4], F32, tag="oclsacc")
                ocls_acc = oscls_acc[:, 0, :]
                scls_acc = oscls_acc[:, 1, :]
                ecc_bc = big.tile([128, 4], F32, tag="eccbc")
                vTcls = big.tile([128, 4], F32, tag="vTcls")
                ecc_row = big.tile([1, H], BF16, tag="eccrow")
                rcls_bc = big.tile([128, 4], F32, tag="rclsbc")
                t_evc = big.tile([128, 4], F32, tag="tevc")
                t_corr = big.tile([128, 4], F32, tag="tcorr")

                nc.vector.memset(oscls_acc, 0.0)

                # ---- Q/K projection (out^T form)
                for oc in range(8):
                    isq = oc < 4
                    hp = oc if isq else oc - 4
                    for ci, (t0, tl) in enumerate(TCH):
                        ps = ps_proj.tile([128, 512], F32, tag="proj")
                        for kc in range(KC):
                            nc.tensor.matmul(
                                ps[:, :tl],
                                lhsT=wqkv_sb[:, kc, oc * 128:(oc + 1) * 128],
                                rhs=xT_sb[:, kc, t0:t0 + tl],
                                start=(kc == 0),
                                stop=(kc == KC - 1),
                            )
                        dst = qT_fr if isq else kT_fr
                        eng = nc.scalar if isq else nc.vector
                        cp = eng.copy if isq else eng.tensor_copy
                        o0 = 0 if isq else 1
                        if ci == 0:
                            cp(
                                dst[:, hp, 0:2, o0:o0 + SP],
                                ps[:, 1:tl].rearrange("p (a s) -> p a s", a=2),
                            )
                            ccol = SP if isq else 0
                            cp(
                                dst[:, hp, 0:F, ccol:ccol + 1],
                                ps[:, None, 0:1].to_broadcast([128, F, 1]),
                            )
                        else:
                            cp(
                                dst[:, hp, 2 * ci:2 * ci + 2, o0:o0 + SP],
                                ps[:, :tl].rearrange("p (a s) -> p a s", a=2),
                            )

                # ---- V projection (natural form) -> v_flat
                for t in range(NT):
                    m = 128 if t < NT - 1 else LAST
                    ps = ps_proj.tile([128, 512], F32, tag="proj")
                    for kc in range(KC):
                        nc.tensor.matmul(
                            ps[:m, :],
                            lhsT=xT_sb[:, kc, 128 * t:128 * t + m],
                            rhs=wqkv_sb[:, kc, 2 * INNER:3 * INNER],
                            start=(kc == 0),
                            stop=(kc == KC - 1),
                        )
                    nc.scalar.copy(v_flat[:m, t, :], ps[:m, :])

                # vT_cls [d x 2heads, hp] via tiny K=1 transposes of the V cls row
                pvt = ps_proj.tile([128, 512], F32, tag="proj")
                for hp in range(4):
                    nc.tensor.matmul(
                        pvt[:, hp:hp + 1],
                        lhsT=v_flat[0:1, 0, 128 * hp:128 * (hp + 1)],
                        rhs=ones_row[0:1, 0:1],
                        start=True, stop=True,
                    )
                nc.vector.tensor_copy(vTcls, pvt[:, 0:4])

                # ---- V rearrange to frame-aligned layouts (sbuf->sbuf DMA)
                nc.sync.dma_start(
                    out=v_fra[0:1, 0:F, :],
                    in_=v_flat[0:1, 0, None, :].to_broadcast([1, F, INNER]),
                )
                for f in range(F):
                    for (t, p0, d0, l) in _v_pieces(1 + SP * f, 127):
                        nc.sync.dma_start(
                            out=v_fra[1 + d0:1 + d0 + l, f, :],
                            in_=v_flat[p0:p0 + l, t, :],
                        )
                    for (t, p0, d0, l) in _v_pieces(128 + SP * f, 69):
                        nc.sync.dma_start(
                            out=v_frb[d0:d0 + l, f, :],
                            in_=v_flat[p0:p0 + l, t, :],
                        )

                # ---- attention (S^T pipeline), globally software-pipelined
                # across head-pair boundaries so the PE always has pass-1
                # matmuls available while pass-2 normalize chains run.
                etiles = {}

                def pass1(hp, f):
                    if f == 0:
                        ea_t = ebuf.tile([128, 2, F, SP1], BF16, tag="ea")
                        eb_t = ebuf.tile([128, 2, F, SP1], BF16, tag="eb")
                        etiles[hp] = (ea_t, eb_t)
                    ea, eb = etiles[hp]
                    st_a = ps_s.tile([128, 2, 512], F32, tag="s")
                    st_b = ps_s.tile([128, 2, 512], F32, tag="s")
                    for par in range(2):
                        rows = slice(64 * par, 64 * par + 64)
                        nc.tensor.matmul(
                            st_a[:, par, 0:SP1],
                            lhsT=kT_fr[rows, hp, f, 0:128],
                            rhs=qT_fr[rows, hp, f, :],
                            start=True, stop=True,
                        )
                        nc.tensor.matmul(
                            st_b[0:69, par, 0:SP1],
                            lhsT=kT_fr[rows, hp, f, 128:SP1],
                            rhs=qT_fr[rows, hp, f, :],
                            start=True, stop=True,
                        )
                    nc.scalar.activation(ea[:, :, f, :], st_a[:, :, 0:SP1], EXP)
                    nc.scalar.activation(eb[0:69, :, f, :], st_b[0:69, :, 0:SP1], EXP)
                    if f == 0:
                        for par in range(2):
                            h = 2 * hp + par
                            nc.scalar.copy(ecc_row[0:1, h:h + 1],
                                           ea[0:1, par, 0, SP:SP1])

                def pass2(hp, f):
                    ea, eb = etiles[hp]
                    po = ps_o.tile([128, 1024], F32, tag="o")
                    for par in range(2):
                        h = 2 * hp + par
                        rows = slice(64 * par, 64 * par + 64)
                        hs = slice(DH * h, DH * (h + 1))
                        nc.tensor.matmul(
                            po[rows, 0:SP1],
                            lhsT=v_fra[:, f, hs],
                            rhs=ea[:, par, f, :],
                            start=True, stop=False,
                        )
                        nc.tensor.matmul(
                            po[rows, 0:SP1],
                            lhsT=v_frb[0:69, f, hs],
                            rhs=eb[0:69, par, f, :],
                            start=False, stop=True,
                        )
                    # column sums broadcast per parity (rows match parity)
                    for par in range(2):
                        rows = slice(64 * par, 64 * par + 64)
                        nc.tensor.matmul(
                            po[rows, 512:512 + SP1],
                            lhsT=ones128[:, 0:64],
                            rhs=ea[:, par, f, :],
                            start=True, stop=False,
                        )
                        nc.tensor.matmul(
                            po[rows, 512:512 + SP1],
                            lhsT=ones128[0:69, 0:64],
                            rhs=eb[0:69, par, f, :],
                            start=False, stop=True,
                        )
                    rbc = small.tile([128, SP1], F32, tag="rbc")
                    nc.vector.reciprocal_approx_fast(rbc, po[:, 512:512 + SP1])
                    nc.vector.tensor_tensor(
                        attnT[:, hp, 1 + SP * f:1 + SP * (f + 1)],
                        po[:, 0:SP], rbc[:, 0:SP], MULT,
                    )
                    nc.vector.tensor_tensor(
                        oscls_acc[:, :, hp], po[:, SP:SP + 513:512],
                        oscls_acc[:, :, hp], ADD,
                    )

                steps = [(hp, f) for hp in range(4) for f in range(F)]
                LAG = 3
                for g in range(len(steps) + LAG):
                    if g < len(steps):
                        pass1(*steps[g])
                    if g >= LAG:
                        pass2(*steps[g - LAG])

                # ---- CLS finalization (bc space)
                pec = ps_proj.tile([128, 512], F32, tag="proj")
                for hp in range(4):
                    for par in range(2):
                        h = 2 * hp + par
                        rows = slice(64 * par, 64 * par + 64)
                        nc.tensor.matmul(pec[rows, hp:hp + 1], lhsT=ones_row,
                                         rhs=ecc_row[0:1, h:h + 1],
                                         start=True, stop=True)
                nc.vector.tensor_copy(ecc_bc, pec[:, 0:4])
                # denominator: scls_acc - 7 e_cc -> reciprocal
                nc.vector.scalar_tensor_tensor(
                    scls_acc, ecc_bc, -7.0, scls_acc, op0=MULT, op1=ADD,
                )
                nc.vector.reciprocal_approx_fast(rcls_bc, scls_acc)
                # numerator: ocls_acc - 7 e_cc * vTcls, then normalize
                nc.vector.tensor_tensor(t_evc, ecc_bc, vTcls, MULT)
                nc.vector.scalar_tensor_tensor(
                    t_corr, t_evc, -7.0, ocls_acc, op0=MULT, op1=ADD,
                )
                nc.vector.tensor_tensor(t_corr, t_corr, rcls_bc, MULT)
                nc.vector.tensor_copy(attnT[:, 0:4, 0:1], t_corr[:, :, None])

                # ---- out projection + bias (token tile 0 last: waits on CLS)
                for t in list(range(1, NT)) + [0]:
                    m = 128 if t < NT - 1 else LAST
                    ps = ps_proj.tile([128, 512], F32, tag="proj")
                    for kc in range(KC):
                        nc.tensor.matmul(
                            ps[:m, :],
                            lhsT=attnT[:, kc, 128 * t:128 * t + m],
                            rhs=wout_sb[:, kc, :],
                            start=(kc == 0),
                            stop=(kc == KC - 1),
                        )
                    osb = outp.tile([128, DIM], F32, tag="out")
                    nc.vector.tensor_tensor(osb[:m, :], ps[:m, :], bout_sb[:m, :], ADD)
                    nc.sync.dma_start(out=out[b, 128 * t:128 * t + m, :], in_=osb[:m, :])

    nc.finalize()
    return nc


_CACHE = {}


def _get_nc():
    if "nc" not in _CACHE:
        _CACHE["nc"] = build_nc()
    return _CACHE["nc"]


def prepare_in_maps(x, f, W_qkv, W_out, b_out):
    assert int(f) == F
    x = np.asarray(x, dtype=np.float32)
    W_qkv = np.asarray(W_qkv, dtype=np.float32).copy()
    W_out = np.asarray(W_out, dtype=np.float32)
    b_out = np.asarray(b_out, dtype=np.float32)
    W_qkv[:, :INNER] *= DH ** -0.5
    wqkv_bf = W_qkv.astype(NPBF)
    wout_bf = W_out.astype(NPBF)
    bout_np = b_out.reshape(1, DIM)
    xT = np.ascontiguousarray(x.transpose(0, 2, 1)).astype(NPBF)
    in_maps = []
    for c in range(NCORES):
        in_maps.append({
            "xT": np.ascontiguousarray(xT[NB * c:NB * (c + 1)]),
            "wqkv": wqkv_bf,
            "wout": wout_bf,
            "bout": bout_np,
        })
    return in_maps


def kernel(x, f, W_qkv, W_out, b_out):
    nc = _get_nc()
    in_maps = prepare_in_maps(x, f, W_qkv, W_out, b_out)
    res = bass_utils.run_bass_kernel_spmd(nc, in_maps, list(range(NCORES)))
    return np.concatenate([r["out"] for r in res.results], axis=0)
